# revision 1
# baseline (speedup 1.0000x reference)
"""Margin-based triplet loss (nn_Criterion) for Trainium2, 8 NeuronCores.

Strategy: anchor-block sharding.  Core c owns anchor rows [512c, 512c+512).
The host buckets triplets by anchor block and converts them into dense
pair-count histograms W_pos/W_neg (pure index preprocessing); the device
computes the full Gram block via PE and reduces the dense weighted margin
losses, so no per-triplet gathers are needed at all.

Math: d(a,b)^2 = |x_a|^2 + |x_b|^2 - 2 x_a.x_b.  Each core owns a block of
512 anchor rows and computes, densely for all (a, b) pairs:
    d = sqrt(relu(n_a + n_b - 2 G) + eps)            (G from PE, bf16)
    A_p = W_pos * (d > c_pos(a));  B_p = A_p * d     (c = beta_label -+ margin,
    A_n = W_neg * (d < c_neg(a));  B_n = A_n * d      W = host pair-count hist)
each as ONE fused DVE scalar_tensor_tensor with accum_out giving the
per-anchor row sums directly.  Final partials:
    pos_sum = sum(B_p) - sum(c_pos * A_p),  pos_cnt = sum(A_p)
    neg_sum = sum(c_neg * A_n) - sum(B_n),  neg_cnt = sum(A_n)
Host combines the 8 cores' partials and divides.
"""

import os

import numpy as np

B, D, T, NCLS = 4096, 1024, 65536, 100
MARGIN = 0.2
EPS = 1e-8
NCORES = 8
P = 128
RB = 512                 # anchor rows per core
NRB = RB // P            # 4 row blocks
KCH = D // P             # 8 contraction chunks
NJ = B // 512            # 8 column tiles of 512
XCH = B // P             # 32 row chunks of X

_COMPILED = None
LAST_RESULTS = None


def _build_nc():
    import concourse.bacc as bacc
    import concourse.bass as bass
    import concourse.mybir as mybir
    import concourse.tile as tile

    f32 = mybir.dt.float32
    bf16 = mybir.dt.bfloat16
    i32 = mybir.dt.int32
    Alu = mybir.AluOpType
    Act = mybir.ActivationFunctionType
    X_AX = mybir.AxisListType.X

    nc = bacc.Bacc("TRN2")

    batch = nc.dram_tensor("batch", [B, D], f32, kind="ExternalInput")
    xa_d = nc.dram_tensor("xa", [RB, D], f32, kind="ExternalInput")
    wpos_d = nc.dram_tensor("wpos", [P, NRB, B], bf16, kind="ExternalInput")
    wneg_d = nc.dram_tensor("wneg", [P, NRB, B], bf16, kind="ExternalInput")
    beta_d = nc.dram_tensor("beta", [NCLS + 1, 1], f32, kind="ExternalInput")
    labrows_d = nc.dram_tensor("labrows", [P, NRB], i32, kind="ExternalInput")
    out_d = nc.dram_tensor("out", [1, 6], f32, kind="ExternalOutput")

    with tile.TileContext(nc) as tc:
        with (
            tc.tile_pool(name="big", bufs=1) as big,           # persistent
            tc.tile_pool(name="xchunk", bufs=5) as xchunk,
            tc.tile_pool(name="join", bufs=4) as join,
            tc.tile_pool(name="small", bufs=1) as small,
            tc.tile_pool(name="gpsum", bufs=7, space="PSUM") as gpsum,
            tc.tile_pool(name="finpsum", bufs=1, space="PSUM") as finpsum,
        ):
            # persistent buffers — split per column tile / per row block so
            # the (conservative) tile-granular dependency tracking matches
            # exactly the transposes each consumer really needs.
            xTj = [
                big.tile([P, KCH, 512], bf16, tag=f"xT{j}", name=f"xT{j}") for j in range(NJ)
            ]
            xaTr = [
                big.tile([P, KCH, P], bf16, tag=f"xaT{r}", name=f"xaT{r}") for r in range(NRB)
            ]
            na_col = small.tile([P, NRB], f32, tag="na")
            # fused-product row-sum accumulators, one column per (r, j)
            APC = small.tile([P, NRB, NJ], f32, tag="APC")
            BPC = small.tile([P, NRB, NJ], f32, tag="BPC")
            ANC = small.tile([P, NRB, NJ], f32, tag="ANC")
            BNC = small.tile([P, NRB, NJ], f32, tag="BNC")

            # ---- prologue: beta / c constants / xa ----
            betaL = small.tile([P, NRB], f32, tag="betaL")
            labrows = small.tile([P, NRB], i32, tag="labrows")
            nc.sync.dma_start(labrows[:], labrows_d[:])
            for q in range(NRB):
                nc.gpsimd.indirect_dma_start(
                    out=betaL[:, q : q + 1],
                    out_offset=None,
                    in_=beta_d[:],
                    in_offset=bass.IndirectOffsetOnAxis(
                        ap=labrows[:, q : q + 1], axis=0
                    ),
                )
            cpos = small.tile([P, NRB], f32, tag="cpos")
            cneg = small.tile([P, NRB], f32, tag="cneg")
            nc.vector.tensor_scalar(cpos[:], betaL[:], -MARGIN, None, Alu.add)
            nc.vector.tensor_scalar(cneg[:], betaL[:], MARGIN, None, Alu.add)
            eps_bias = small.tile([P, 1], f32, tag="eps")
            nc.vector.memset(eps_bias[:], EPS)
            ones1f = small.tile([1, P], f32, tag="ones1f")
            nc.vector.memset(ones1f[:], 1.0)
            onescol = small.tile([P, 1], f32, tag="onescol")
            nc.vector.memset(onescol[:], 1.0)

            for q in range(NRB):
                xcf = xchunk.tile([P, D], f32, tag="xcf")
                nc.sync.dma_start(xcf[:], xa_d[q * P : (q + 1) * P, :])
                xc = xchunk.tile([P, D], bf16, tag="xc")
                nc.vector.tensor_copy(xc[:], xcf[:])
                sc = xchunk.tile([P, D], bf16, tag="sc")
                nc.scalar.activation(
                    sc[:], xc[:], Act.Square, accum_out=na_col[:, q : q + 1]
                )
                nc.sync.dma_start_transpose(xaTr[q][:], xc[:])

            # ---- pipelined main loop over column tiles j ----
            # Iteration j: cast-load X chunks 4j..4j+3, square-accum each into
            # a per-chunk norm column, flatten those into the per-j (-n_b/2)
            # row piece, transpose the chunks into xT; then the 4 anchor
            # row-block joins for column tile j.  Every instruction of
            # iteration j depends only on iteration-j data plus the prologue,
            # so PE streams continuously instead of waiting for the loads.
            for j in range(NJ):
                ncols = slice(j * 512, (j + 1) * 512)
                nbm_j = join.tile([1, 512], f32, tag="nbm")
                wp_j = join.tile([P, NRB, 512], bf16, tag="wp")
                wn_j = join.tile([P, NRB, 512], bf16, tag="wn")
                nc.scalar.dma_start(wp_j[:], wpos_d[:, :, ncols])
                nc.scalar.dma_start(wn_j[:], wneg_d[:, :, ncols])
                for kk in range(4):
                    k = 4 * j + kk
                    xcf = xchunk.tile([P, D], f32, tag="xcf")
                    nc.sync.dma_start(xcf[:], batch[k * P : (k + 1) * P, :])
                    xc = xchunk.tile([P, D], bf16, tag="xc")
                    if kk % 2 == 0:
                        nc.vector.tensor_copy(xc[:], xcf[:])
                    else:
                        nc.scalar.copy(xc[:], xcf[:])
                    sc = xchunk.tile([P, D], bf16, tag="sc")
                    nck = xchunk.tile([P, 1], f32, tag="nck")
                    nc.scalar.activation(
                        sc[:], xc[:], Act.Square, accum_out=nck[:]
                    )
                    nckm = xchunk.tile([P, 1], f32, tag="nckm")
                    nc.vector.tensor_scalar(nckm[:], nck[:], -0.5, None, Alu.mult)
                    nc.sync.dma_start(
                        nbm_j[0:1, kk * P : (kk + 1) * P], nckm[:]
                    )
                    nc.sync.dma_start_transpose(
                        xTj[j][:, :, kk * P : (kk + 1) * P], xc[:]
                    )

                for r in range(NRB):
                    g = gpsum.tile([P, 512], f32, tag="g", space="PSUM")
                    for i in range(KCH):
                        nc.tensor.matmul(
                            g[:],
                            xaTr[r][:, i, :],
                            xTj[j][:, i, :],
                            start=(i == 0),
                            stop=False,
                        )
                    # n_b contribution: k=1 fp32 matmul ones^T @ (-n_b/2)
                    nc.tensor.matmul(
                        g[:], ones1f[:], nbm_j[:], start=False, stop=True
                    )
                    u = join.tile([P, 512], f32, tag="u")
                    nc.scalar.activation(
                        u[:], g[:], Act.Relu, bias=na_col[:, r : r + 1], scale=-2.0
                    )
                    d = join.tile([P, 512], bf16, tag="d")
                    nc.scalar.activation(d[:], u[:], Act.Sqrt, bias=eps_bias[:])

                    # fused mask*weight (and *d) products with row-sum accum
                    Ap = join.tile([P, 512], bf16, tag="Ap")
                    An = join.tile([P, 512], bf16, tag="An")
                    sc2 = join.tile([P, 512], bf16, tag="sc2")
                    nc.vector.scalar_tensor_tensor(
                        Ap[:], d[:], cpos[:, r : r + 1], wp_j[:, r, :],
                        Alu.is_gt, Alu.mult,
                        accum_out=APC[:, r, j : j + 1],
                    )
                    nc.vector.scalar_tensor_tensor(
                        sc2[:], d[:], 1.0, Ap[:], Alu.mult, Alu.mult,
                        accum_out=BPC[:, r, j : j + 1],
                    )
                    nc.vector.scalar_tensor_tensor(
                        An[:], d[:], cneg[:, r : r + 1], wn_j[:, r, :],
                        Alu.is_lt, Alu.mult,
                        accum_out=ANC[:, r, j : j + 1],
                    )
                    nc.vector.scalar_tensor_tensor(
                        sc2[:], d[:], 1.0, An[:], Alu.mult, Alu.mult,
                        accum_out=BNC[:, r, j : j + 1],
                    )

            # ---- finale: tiny reductions + one partition-sum matmul ----
            tA = small.tile([P, NRB], f32, tag="tA")
            tB = small.tile([P, NRB], f32, tag="tB")
            tAn = small.tile([P, NRB], f32, tag="tAn")
            tBn = small.tile([P, NRB], f32, tag="tBn")
            nc.vector.tensor_reduce(tA[:], APC[:], X_AX, Alu.add)
            nc.vector.tensor_reduce(tB[:], BPC[:], X_AX, Alu.add)
            nc.vector.tensor_reduce(tAn[:], ANC[:], X_AX, Alu.add)
            nc.vector.tensor_reduce(tBn[:], BNC[:], X_AX, Alu.add)
            cA = small.tile([P, NRB], f32, tag="cA")
            cN = small.tile([P, NRB], f32, tag="cN")
            nc.vector.tensor_tensor(cA[:], tA[:], cpos[:], Alu.mult)
            nc.vector.tensor_tensor(cN[:], tAn[:], cneg[:], Alu.mult)
            F = small.tile([P, 6], f32, tag="F")
            nc.vector.tensor_reduce(F[:, 0:1], tA[:], X_AX, Alu.add)
            nc.vector.tensor_reduce(F[:, 1:2], cA[:], X_AX, Alu.add)
            nc.vector.tensor_reduce(F[:, 2:3], tB[:], X_AX, Alu.add)
            nc.vector.tensor_reduce(F[:, 3:4], tAn[:], X_AX, Alu.add)
            nc.vector.tensor_reduce(F[:, 4:5], cN[:], X_AX, Alu.add)
            nc.vector.tensor_reduce(F[:, 5:6], tBn[:], X_AX, Alu.add)
            fin = finpsum.tile([1, 6], f32, tag="fin", space="PSUM")
            nc.tensor.matmul(fin[:], onescol[:], F[:], start=True, stop=True)
            out_sb = small.tile([1, 6], f32, tag="out_sb")
            nc.vector.tensor_copy(out_sb[:], fin[:])
            nc.sync.dma_start(out_d[:], out_sb[:])

    nc.compile()
    return nc


def _prep_inputs(batch, labels, triplets, beta):
    import ml_dtypes

    bf = ml_dtypes.bfloat16
    trip = np.asarray(triplets).astype(np.int64)
    labs = np.asarray(labels).astype(np.int32)
    batch = np.ascontiguousarray(np.asarray(batch), dtype=np.float32)
    beta_in = np.ascontiguousarray(
        np.asarray(beta), dtype=np.float32
    ).reshape(NCLS + 1, 1)

    in_maps = []
    for c in range(NCORES):
        lo, hi = c * RB, (c + 1) * RB
        sel = (trip[:, 0] >= lo) & (trip[:, 0] < hi)
        t = trip[sel]
        a_loc = t[:, 0] - lo
        wpos = np.bincount(a_loc * B + t[:, 1], minlength=RB * B).reshape(RB, B)
        wneg = np.bincount(a_loc * B + t[:, 2], minlength=RB * B).reshape(RB, B)

        def togrid(w):
            return np.ascontiguousarray(
                w.reshape(NRB, P, B).transpose(1, 0, 2)
            ).astype(bf)

        labrows = np.ascontiguousarray(
            labs[lo:hi].reshape(NRB, P).T
        ).astype(np.int32)
        in_maps.append(
            {
                "batch": batch,
                "xa": np.ascontiguousarray(batch[lo:hi]),
                "wpos": togrid(wpos),
                "wneg": togrid(wneg),
                "beta": beta_in,
                "labrows": labrows,
            }
        )
    return in_maps


def kernel(batch, labels, triplets, beta):
    global _COMPILED, LAST_RESULTS
    from concourse.bass_utils import run_bass_kernel_spmd

    if _COMPILED is None:
        _COMPILED = _build_nc()
    nc = _COMPILED

    in_maps = _prep_inputs(batch, labels, triplets, beta)
    trace = bool(int(os.environ.get("KERNEL_TRACE", "0")))
    res = run_bass_kernel_spmd(
        nc, in_maps, core_ids=list(range(NCORES)), trace=trace
    )
    LAST_RESULTS = res

    pos_sum = neg_sum = cnt = 0.0
    for r in res.results:
        o = r["out"].astype(np.float64).ravel()
        cntP, cPA, sBp, cntN, cNA, sBn = o[0], o[1], o[2], o[3], o[4], o[5]
        pos_sum += sBp - cPA
        neg_sum += cNA - sBn
        cnt += cntP + cntN
    total = pos_sum + neg_sum
    loss = total if cnt == 0.0 else total / cnt
    return np.float32(loss)



# revision 11
# speedup vs baseline: 3.1817x; 3.1817x over previous
"""Margin-based triplet loss (nn_Criterion) for Trainium2, 8 NeuronCores.

Fast path ("screened" kernel): anchor-block data-parallel over 8 cores.
Core c owns anchors [512c, 512c+512).  Host preprocessing (pure index /
layout work, same category as the original bincount histograms):
  * casts batch to fp8e4 and ships it pre-transposed (contraction dim on
    partitions), so the device does zero transposes,
  * ships -|x|^2/2 norm rows (computed from the fp8-rounded batch, so
    d^2(a,a) == 0 exactly up to bf16 rounding),
  * excludes index-duplicate pairs (triplet with p==a or n==a) from the
    pos histogram W; those pairs have d = sqrt(eps) analytically and are
    corrected on the host.

Device per core: PSUM units of [128 anchors x 1024 cols]:
  psum = Xa_fp8 @ X_fp8^T  (DoubleRow fp8 matmuls, k=1024)
         + ones (x) (-n_b/2) + (-n_a/2) (x) ones   (one k=2 bf16 matmul)
  u = Relu(-2 psum)        == d^2              (ACT)
  sig += sum 1[u <= 100]   (TS 4x, accum)      -- the screen
  d = Sqrt(u + eps)                            (ACT)
  v = W (*) d  (TT 2x);  swd += sum v  (TS 4x, accum)

The margin thresholds (beta -+ margin ~= 1.0/1.4) can only bind where
d <~ 10, and the screen counts exactly those pairs: it must find exactly
one (the self-pair b==a) per anchor row.  If the screen finds anything
else, the data violates the far-pair assumption and kernel() reruns with
the exact (slower) kernel below.  For the reference input the screen
always passes, so the hot path is:
  loss = (sum W*d  - sum_nondup c_pos(t) + host dup corrections) / count.

Exact fallback: the original dense-masked kernel (relu/sqrt + 4 masked
DVE reductions per tile), bit-identical semantics to the reference.
"""

import os

import numpy as np

B, D, T, NCLS = 4096, 1024, 65536, 100
MARGIN = 0.2
EPS = 1e-8
NCORES = 8
P = 128
RB = 512                 # anchor rows per core
NRB = RB // P            # 4 anchor row blocks (r)
NH = 4                   # column quarter tiles of 1024 (h)
CW = B // NH             # 1024 cols per unit
NU = NRB * NH            # 16 units per core
KCH = D // P             # 8 contraction chunks
SCREEN_T = 100.0         # u = d^2 screen threshold (d <= 10)

_COMPILED_FAST = None
_COMPILED_EXACT = None
LAST_RESULTS = None
USE_DR = True            # fp8 DoubleRow matmuls


# ---------------------------------------------------------------------------
# fast (screened) kernel
# ---------------------------------------------------------------------------

def _build_fast():
    import concourse.bacc as bacc
    import concourse.bass as bass
    import concourse.mybir as mybir
    import concourse.tile as tile

    f32 = mybir.dt.float32
    bf16 = mybir.dt.bfloat16
    fp8 = mybir.dt.float8e4
    Alu = mybir.AluOpType
    Act = mybir.ActivationFunctionType

    nc = bacc.Bacc("TRN2")

    # xt: [p, 8h + i, 1024] = X^T fp8 grouped by column quarter
    xt_d = nc.dram_tensor("xt", [P, NH * KCH, CW], fp8, kind="ExternalInput")
    xa_d = nc.dram_tensor("xa", [P, KCH, RB], fp8, kind="ExternalInput")
    w_d = nc.dram_tensor("w", [P, NU, CW], bf16, kind="ExternalInput")
    r2_d = nc.dram_tensor("r2", [2, B], bf16, kind="ExternalInput")
    l2_d = nc.dram_tensor("l2", [2, RB], bf16, kind="ExternalInput")
    out_d = nc.dram_tensor("out", [P, 2 * NU], f32, kind="ExternalOutput")

    with tile.TileContext(nc) as tc:
        with (
            tc.tile_pool(name="big", bufs=1) as big,
            tc.tile_pool(name="small", bufs=1) as small,
            tc.tile_pool(name="up", bufs=3) as up,
            tc.tile_pool(name="dp", bufs=3) as dp,
            tc.tile_pool(name="vp", bufs=2) as vp,
            tc.tile_pool(name="jp", bufs=2) as jp,
            tc.tile_pool(name="j2p", bufs=2) as j2p,
            tc.tile_pool(name="gpsum", bufs=4, space="PSUM") as gpsum,
        ):
            xa = big.tile([P, KCH, RB], fp8, tag="xa")
            nc.sync.dma_start(xa[:], xa_d[:])
            xts = [
                big.tile([P, KCH, CW], fp8, tag=f"xt{h}", name=f"xt{h}")
                for h in range(NH)
            ]
            wrs = [
                big.tile([P, NH, CW], bf16, tag=f"w{r}", name=f"w{r}")
                for r in range(NRB)
            ]
            # interleave the big loads so unit (r=0, h=0) can start early
            for h in range(NH):
                nc.sync.dma_start(xts[h][:], xt_d[:, h * KCH : (h + 1) * KCH, :])
                nc.scalar.dma_start(wrs[h][:], w_d[:, h * NH : (h + 1) * NH, :])

            # norm rows for the k=2 bf16 matmul:
            #   L2 = [ones ; -n_a/2]  (k x anchors), R2 = [-n_b/2 ; ones]
            L2 = small.tile([2, RB], bf16, tag="L2")
            R2 = small.tile([2, B], bf16, tag="R2")
            nc.sync.dma_start(L2[:], l2_d[:])
            nc.sync.dma_start(R2[:], r2_d[:])
            epsb = small.tile([P, 1], f32, tag="eps")
            nc.vector.memset(epsb[:], EPS)
            OUT = small.tile([P, 2 * NU], f32, tag="OUT")

            for r in range(NRB):
                psums = [
                    gpsum.tile([P, CW], f32, tag="g", space="PSUM", name=f"g{r}_{h}")
                    for h in range(NH)
                ]
                acol = slice(r * P, (r + 1) * P)
                # k=2 norm matmul initializes each accumulation group
                for h in range(NH):
                    for q in range(2):
                        cs = h * CW + q * 512
                        nc.tensor.matmul(
                            psums[h][:, q * 512 : (q + 1) * 512],
                            L2[:, acol],
                            R2[:, cs : cs + 512],
                            start=True,
                            stop=False,
                        )
                if USE_DR:
                    for i in range(KCH // 2):
                        lhs = xa[:, 2 * i : 2 * i + 2, acol]
                        for h in range(NH):
                            for q in range(2):
                                nc.tensor.matmul(
                                    psums[h][:, q * 512 : (q + 1) * 512],
                                    lhs,
                                    xts[h][:, 2 * i : 2 * i + 2, q * 512 : (q + 1) * 512],
                                    start=False,
                                    stop=(i == KCH // 2 - 1),
                                    perf_mode=mybir.MatmulPerfMode.DoubleRow,
                                )
                else:
                    for i in range(KCH):
                        lhs = xa[:, i, acol]
                        for h in range(NH):
                            for q in range(2):
                                nc.tensor.matmul(
                                    psums[h][:, q * 512 : (q + 1) * 512],
                                    lhs,
                                    xts[h][:, i, q * 512 : (q + 1) * 512],
                                    start=False,
                                    stop=(i == KCH - 1),
                                )
                for h in range(NH):
                    u_idx = r * NH + h
                    u = up.tile([P, CW], bf16, tag="u")
                    nc.scalar.activation(u[:], psums[h][:], Act.Relu, scale=-2.0)
                    j2 = j2p.tile([P, CW], bf16, tag="j2")
                    nc.vector.tensor_scalar(
                        j2[:], u[:], SCREEN_T, None, Alu.is_le, Alu.add,
                        accum_out=OUT[:, NU + u_idx : NU + u_idx + 1],
                    )
                    d = dp.tile([P, CW], bf16, tag="d")
                    nc.scalar.activation(d[:], u[:], Act.Sqrt, bias=epsb[:])
                    v = vp.tile([P, CW], bf16, tag="v")
                    nc.vector.tensor_tensor(
                        v[:], wrs[r][:, h, :], d[:], Alu.mult
                    )
                    j = jp.tile([P, CW], bf16, tag="j")
                    nc.vector.tensor_scalar(
                        j[:], v[:], 1.0, None, Alu.mult, Alu.add,
                        accum_out=OUT[:, u_idx : u_idx + 1],
                    )

            nc.sync.dma_start(out_d[:], OUT[:])

    nc.compile()
    return nc


def _prep_fast(batch, labels, triplets, beta):
    import concourse.mybir as mybir
    import ml_dtypes

    bfnp = ml_dtypes.bfloat16
    fp8np = mybir.dt.np(mybir.dt.float8e4)

    X = np.ascontiguousarray(np.asarray(batch), dtype=np.float32)
    trip = np.asarray(triplets).astype(np.int64)

    X8 = X.astype(fp8np)
    X8f = X8.astype(np.float64)
    nrm = (X8f * X8f).sum(axis=1)           # fp8-consistent norms, f64
    r2 = np.ascontiguousarray(
        np.stack([(-0.5 * nrm), np.ones(B)]).astype(bfnp)
    )  # [-n_b/2 ; ones]

    XTfull = np.ascontiguousarray(
        X8.T.reshape(KCH, P, B).transpose(1, 0, 2)
    )  # [p, i, b]
    xtd = np.ascontiguousarray(
        np.concatenate(
            [XTfull[:, :, h * CW : (h + 1) * CW] for h in range(NH)], axis=1
        )
    )  # [p, 8h+i, 1024]

    a = trip[:, 0]
    p_ = trip[:, 1]
    nondup = a != p_

    in_maps = []
    for c in range(NCORES):
        lo, hi = c * RB, (c + 1) * RB
        xaT = np.ascontiguousarray(
            X8[lo:hi].T.reshape(KCH, P, RB).transpose(1, 0, 2)
        )
        l2 = np.ascontiguousarray(
            np.stack([np.ones(RB), (-0.5 * nrm[lo:hi])]).astype(bfnp)
        )  # [ones ; -n_a/2]
        sel = nondup & (a >= lo) & (a < hi)
        W = np.bincount(
            (a[sel] - lo) * B + p_[sel], minlength=RB * B
        ).reshape(RB, B)
        if W.max() > 255:
            raise OverflowError("W count exceeds bf16-exact range")
        wdev = np.ascontiguousarray(
            W.reshape(NRB, P, NH, CW).transpose(1, 0, 2, 3).reshape(P, NU, CW)
        ).astype(bfnp)
        in_maps.append(
            {"xt": xtd, "xa": xaT, "w": wdev, "r2": r2, "l2": l2}
        )
    return in_maps


def _host_loss_fast(res, labels, triplets, beta):
    """Combine device partials with host dup corrections. Returns
    (loss, screen_ok)."""
    trip = np.asarray(triplets).astype(np.int64)
    labs = np.asarray(labels).astype(np.int64)
    bet = np.asarray(beta).astype(np.float64)

    swd = 0.0
    screen_ok = True
    for r in res.results:
        o = r["out"].astype(np.float64)
        sw, sig = o[:, :NU], o[:, NU:]
        if not np.isfinite(sw).all() or not np.isfinite(sig).all():
            screen_ok = False
            break
        swd += sw.sum()
        # per (p, r): self-pair count across the 4 column quarters == 1
        per_anchor = sig.reshape(P, NRB, NH).sum(axis=2)
        if not np.all(per_anchor == 1.0):
            screen_ok = False
            break
    if not screen_ok:
        return None, False

    a, p_, n_ = trip[:, 0], trip[:, 1], trip[:, 2]
    c_pos = bet[labs[a]] - MARGIN           # per-triplet thresholds
    c_neg = bet[labs[a]] + MARGIN
    d_dup = float(np.sqrt(EPS))

    dup_p = a == p_
    dup_n = a == n_

    # pos: nondup pairs are all active (screen-verified): relu -> d - c_pos
    pos_sum = swd - c_pos[~dup_p].sum()
    pos_cnt = float((~dup_p).sum())
    ld = np.maximum(d_dup - c_pos[dup_p], 0.0)
    pos_sum += ld.sum()
    pos_cnt += float((ld > 0).sum())

    # neg: nondup pairs are all inactive (screen-verified)
    ln = np.maximum(c_neg[dup_n] - d_dup, 0.0)
    neg_sum = ln.sum()
    neg_cnt = float((ln > 0).sum())

    total = pos_sum + neg_sum
    cnt = pos_cnt + neg_cnt
    loss = total if cnt == 0.0 else total / cnt
    return np.float32(loss), True


# ---------------------------------------------------------------------------
# exact fallback kernel (original dense-masked implementation)
# ---------------------------------------------------------------------------

RB_E = 512
NRB_E = RB_E // P
KCH_E = D // P
NJ_E = B // 512
XCH_E = B // P


def _build_exact():
    import concourse.bacc as bacc
    import concourse.bass as bass
    import concourse.mybir as mybir
    import concourse.tile as tile

    f32 = mybir.dt.float32
    bf16 = mybir.dt.bfloat16
    i32 = mybir.dt.int32
    Alu = mybir.AluOpType
    Act = mybir.ActivationFunctionType
    X_AX = mybir.AxisListType.X

    nc = bacc.Bacc("TRN2")

    batch = nc.dram_tensor("batch", [B, D], f32, kind="ExternalInput")
    xa_d = nc.dram_tensor("xa", [RB_E, D], f32, kind="ExternalInput")
    wpos_d = nc.dram_tensor("wpos", [P, NRB_E, B], bf16, kind="ExternalInput")
    wneg_d = nc.dram_tensor("wneg", [P, NRB_E, B], bf16, kind="ExternalInput")
    beta_d = nc.dram_tensor("beta", [NCLS + 1, 1], f32, kind="ExternalInput")
    labrows_d = nc.dram_tensor("labrows", [P, NRB_E], i32, kind="ExternalInput")
    out_d = nc.dram_tensor("out", [1, 6], f32, kind="ExternalOutput")

    with tile.TileContext(nc) as tc:
        with (
            tc.tile_pool(name="big", bufs=1) as big,
            tc.tile_pool(name="xchunk", bufs=5) as xchunk,
            tc.tile_pool(name="join", bufs=4) as join,
            tc.tile_pool(name="small", bufs=1) as small,
            tc.tile_pool(name="gpsum", bufs=7, space="PSUM") as gpsum,
            tc.tile_pool(name="finpsum", bufs=1, space="PSUM") as finpsum,
        ):
            xTj = [
                big.tile([P, KCH_E, 512], bf16, tag=f"xT{j}", name=f"xT{j}")
                for j in range(NJ_E)
            ]
            xaTr = [
                big.tile([P, KCH_E, P], bf16, tag=f"xaT{r}", name=f"xaT{r}")
                for r in range(NRB_E)
            ]
            na_col = small.tile([P, NRB_E], f32, tag="na")
            APC = small.tile([P, NRB_E, NJ_E], f32, tag="APC")
            BPC = small.tile([P, NRB_E, NJ_E], f32, tag="BPC")
            ANC = small.tile([P, NRB_E, NJ_E], f32, tag="ANC")
            BNC = small.tile([P, NRB_E, NJ_E], f32, tag="BNC")

            betaL = small.tile([P, NRB_E], f32, tag="betaL")
            labrows = small.tile([P, NRB_E], i32, tag="labrows")
            nc.sync.dma_start(labrows[:], labrows_d[:])
            for q in range(NRB_E):
                nc.gpsimd.indirect_dma_start(
                    out=betaL[:, q : q + 1],
                    out_offset=None,
                    in_=beta_d[:],
                    in_offset=bass.IndirectOffsetOnAxis(
                        ap=labrows[:, q : q + 1], axis=0
                    ),
                )
            cpos = small.tile([P, NRB_E], f32, tag="cpos")
            cneg = small.tile([P, NRB_E], f32, tag="cneg")
            nc.vector.tensor_scalar(cpos[:], betaL[:], -MARGIN, None, Alu.add)
            nc.vector.tensor_scalar(cneg[:], betaL[:], MARGIN, None, Alu.add)
            eps_bias = small.tile([P, 1], f32, tag="eps")
            nc.vector.memset(eps_bias[:], EPS)
            ones1f = small.tile([1, P], f32, tag="ones1f")
            nc.vector.memset(ones1f[:], 1.0)
            onescol = small.tile([P, 1], f32, tag="onescol")
            nc.vector.memset(onescol[:], 1.0)

            for q in range(NRB_E):
                xcf = xchunk.tile([P, D], f32, tag="xcf")
                nc.sync.dma_start(xcf[:], xa_d[q * P : (q + 1) * P, :])
                xc = xchunk.tile([P, D], bf16, tag="xc")
                nc.vector.tensor_copy(xc[:], xcf[:])
                sc = xchunk.tile([P, D], bf16, tag="sc")
                nc.scalar.activation(
                    sc[:], xc[:], Act.Square, accum_out=na_col[:, q : q + 1]
                )
                nc.sync.dma_start_transpose(xaTr[q][:], xc[:])

            for j in range(NJ_E):
                ncols = slice(j * 512, (j + 1) * 512)
                nbm_j = join.tile([1, 512], f32, tag="nbm")
                wp_j = join.tile([P, NRB_E, 512], bf16, tag="wp")
                wn_j = join.tile([P, NRB_E, 512], bf16, tag="wn")
                nc.scalar.dma_start(wp_j[:], wpos_d[:, :, ncols])
                nc.scalar.dma_start(wn_j[:], wneg_d[:, :, ncols])
                for kk in range(4):
                    k = 4 * j + kk
                    xcf = xchunk.tile([P, D], f32, tag="xcf")
                    nc.sync.dma_start(xcf[:], batch[k * P : (k + 1) * P, :])
                    xc = xchunk.tile([P, D], bf16, tag="xc")
                    if kk % 2 == 0:
                        nc.vector.tensor_copy(xc[:], xcf[:])
                    else:
                        nc.scalar.copy(xc[:], xcf[:])
                    sc = xchunk.tile([P, D], bf16, tag="sc")
                    nck = xchunk.tile([P, 1], f32, tag="nck")
                    nc.scalar.activation(
                        sc[:], xc[:], Act.Square, accum_out=nck[:]
                    )
                    nckm = xchunk.tile([P, 1], f32, tag="nckm")
                    nc.vector.tensor_scalar(nckm[:], nck[:], -0.5, None, Alu.mult)
                    nc.sync.dma_start(
                        nbm_j[0:1, kk * P : (kk + 1) * P], nckm[:]
                    )
                    nc.sync.dma_start_transpose(
                        xTj[j][:, :, kk * P : (kk + 1) * P], xc[:]
                    )

                for r in range(NRB_E):
                    g = gpsum.tile([P, 512], f32, tag="g", space="PSUM")
                    for i in range(KCH_E):
                        nc.tensor.matmul(
                            g[:],
                            xaTr[r][:, i, :],
                            xTj[j][:, i, :],
                            start=(i == 0),
                            stop=False,
                        )
                    nc.tensor.matmul(
                        g[:], ones1f[:], nbm_j[:], start=False, stop=True
                    )
                    u = join.tile([P, 512], f32, tag="u")
                    nc.scalar.activation(
                        u[:], g[:], Act.Relu, bias=na_col[:, r : r + 1], scale=-2.0
                    )
                    d = join.tile([P, 512], bf16, tag="d")
                    nc.scalar.activation(d[:], u[:], Act.Sqrt, bias=eps_bias[:])

                    Ap = join.tile([P, 512], bf16, tag="Ap")
                    An = join.tile([P, 512], bf16, tag="An")
                    sc2 = join.tile([P, 512], bf16, tag="sc2")
                    nc.vector.scalar_tensor_tensor(
                        Ap[:], d[:], cpos[:, r : r + 1], wp_j[:, r, :],
                        Alu.is_gt, Alu.mult,
                        accum_out=APC[:, r, j : j + 1],
                    )
                    nc.vector.scalar_tensor_tensor(
                        sc2[:], d[:], 1.0, Ap[:], Alu.mult, Alu.mult,
                        accum_out=BPC[:, r, j : j + 1],
                    )
                    nc.vector.scalar_tensor_tensor(
                        An[:], d[:], cneg[:, r : r + 1], wn_j[:, r, :],
                        Alu.is_lt, Alu.mult,
                        accum_out=ANC[:, r, j : j + 1],
                    )
                    nc.vector.scalar_tensor_tensor(
                        sc2[:], d[:], 1.0, An[:], Alu.mult, Alu.mult,
                        accum_out=BNC[:, r, j : j + 1],
                    )

            tA = small.tile([P, NRB_E], f32, tag="tA")
            tB = small.tile([P, NRB_E], f32, tag="tB")
            tAn = small.tile([P, NRB_E], f32, tag="tAn")
            tBn = small.tile([P, NRB_E], f32, tag="tBn")
            nc.vector.tensor_reduce(tA[:], APC[:], X_AX, Alu.add)
            nc.vector.tensor_reduce(tB[:], BPC[:], X_AX, Alu.add)
            nc.vector.tensor_reduce(tAn[:], ANC[:], X_AX, Alu.add)
            nc.vector.tensor_reduce(tBn[:], BNC[:], X_AX, Alu.add)
            cA = small.tile([P, NRB_E], f32, tag="cA")
            cN = small.tile([P, NRB_E], f32, tag="cN")
            nc.vector.tensor_tensor(cA[:], tA[:], cpos[:], Alu.mult)
            nc.vector.tensor_tensor(cN[:], tAn[:], cneg[:], Alu.mult)
            F = small.tile([P, 6], f32, tag="F")
            nc.vector.tensor_reduce(F[:, 0:1], tA[:], X_AX, Alu.add)
            nc.vector.tensor_reduce(F[:, 1:2], cA[:], X_AX, Alu.add)
            nc.vector.tensor_reduce(F[:, 2:3], tB[:], X_AX, Alu.add)
            nc.vector.tensor_reduce(F[:, 3:4], tAn[:], X_AX, Alu.add)
            nc.vector.tensor_reduce(F[:, 4:5], cN[:], X_AX, Alu.add)
            nc.vector.tensor_reduce(F[:, 5:6], tBn[:], X_AX, Alu.add)
            fin = finpsum.tile([1, 6], f32, tag="fin", space="PSUM")
            nc.tensor.matmul(fin[:], onescol[:], F[:], start=True, stop=True)
            out_sb = small.tile([1, 6], f32, tag="out_sb")
            nc.vector.tensor_copy(out_sb[:], fin[:])
            nc.sync.dma_start(out_d[:], out_sb[:])

    nc.compile()
    return nc


def _prep_exact(batch, labels, triplets, beta):
    import ml_dtypes

    bf = ml_dtypes.bfloat16
    trip = np.asarray(triplets).astype(np.int64)
    labs = np.asarray(labels).astype(np.int32)
    batch = np.ascontiguousarray(np.asarray(batch), dtype=np.float32)
    beta_in = np.ascontiguousarray(
        np.asarray(beta), dtype=np.float32
    ).reshape(NCLS + 1, 1)

    in_maps = []
    for c in range(NCORES):
        lo, hi = c * RB_E, (c + 1) * RB_E
        sel = (trip[:, 0] >= lo) & (trip[:, 0] < hi)
        t = trip[sel]
        a_loc = t[:, 0] - lo
        wpos = np.bincount(a_loc * B + t[:, 1], minlength=RB_E * B).reshape(RB_E, B)
        wneg = np.bincount(a_loc * B + t[:, 2], minlength=RB_E * B).reshape(RB_E, B)

        def togrid(w):
            return np.ascontiguousarray(
                w.reshape(NRB_E, P, B).transpose(1, 0, 2)
            ).astype(bf)

        labrows = np.ascontiguousarray(
            labs[lo:hi].reshape(NRB_E, P).T
        ).astype(np.int32)
        in_maps.append(
            {
                "batch": batch,
                "xa": np.ascontiguousarray(batch[lo:hi]),
                "wpos": togrid(wpos),
                "wneg": togrid(wneg),
                "beta": beta_in,
                "labrows": labrows,
            }
        )
    return in_maps


def _run_exact(batch, labels, triplets, beta, trace):
    global _COMPILED_EXACT, LAST_RESULTS
    from concourse.bass_utils import run_bass_kernel_spmd

    if _COMPILED_EXACT is None:
        _COMPILED_EXACT = _build_exact()
    in_maps = _prep_exact(batch, labels, triplets, beta)
    res = run_bass_kernel_spmd(
        _COMPILED_EXACT, in_maps, core_ids=list(range(NCORES)), trace=trace
    )
    LAST_RESULTS = res

    pos_sum = neg_sum = cnt = 0.0
    for r in res.results:
        o = r["out"].astype(np.float64).ravel()
        cntP, cPA, sBp, cntN, cNA, sBn = o[0], o[1], o[2], o[3], o[4], o[5]
        pos_sum += sBp - cPA
        neg_sum += cNA - sBn
        cnt += cntP + cntN
    total = pos_sum + neg_sum
    loss = total if cnt == 0.0 else total / cnt
    return np.float32(loss)


# ---------------------------------------------------------------------------


def kernel(batch, labels, triplets, beta):
    global _COMPILED_FAST, LAST_RESULTS
    from concourse.bass_utils import run_bass_kernel_spmd

    trace = bool(int(os.environ.get("KERNEL_TRACE", "0")))
    if os.environ.get("KERNEL_FORCE_EXACT"):
        return _run_exact(batch, labels, triplets, beta, trace)

    try:
        if _COMPILED_FAST is None:
            _COMPILED_FAST = _build_fast()
        in_maps = _prep_fast(batch, labels, triplets, beta)
        res = run_bass_kernel_spmd(
            _COMPILED_FAST, in_maps, core_ids=list(range(NCORES)), trace=trace
        )
        LAST_RESULTS = res
        loss, ok = _host_loss_fast(res, labels, triplets, beta)
    except OverflowError:
        ok = False
    if not ok:
        return _run_exact(batch, labels, triplets, beta, trace)
    return loss
